# revision 11
# baseline (speedup 1.0000x reference)
"""Trainium2 Bass kernel for AttentiveRelationalModuleUniformObs, v5.

Math (per sample b over N=256 neighbors, D=64, LAT=128, EC=32):
    feat   = relu(nbr @ Wf + bf)            [N, LAT]
    enc    = relu(nbr @ Wc + bc)            [N, EC]
    att    = softmax_N(enc @ Wa2)           [N, LAT]   (self/mean/ba cancel)
    out[b] = relu((att * feat).sum(N) @ Wl + bl)

Strategy: transposed per-sample layout [LAT on partitions, N on free],
4 samples per group, data-parallel over 8 cores (128 samples/core).
bf16 everywhere except PSUM and accumulators (HW rel err ~1.1e-3).

Engine plan per group (hard ISA limits: GPSIMD cannot touch PSUM and
cannot run accum_out/TensorScalarPtr, so PSUM drains live on ACT/DVE
and all accum reductions on DVE; DVE pays ~95ns pipeline-drain per op,
so op COUNT is what matters there):
  PE   : enc 2x K=128 blockdiag matmuls -> C [128,256] (4 samples' EC
         stacked on partitions, software-pipelined ONE GROUP AHEAD with
         C manually double-buffered inside one PSUM bank), att 4x K=64
         (Wa2 in lhsT rows 32j), feat 4x K=64.
  ACT  : FR1 = relu(F1+bf) FIRST (needs no E; frees the F buffer so
         the next group's feat matmuls aren't blocked); then one
         3-sample batched exp + a 1-sample exp whose accum_out gives
         den3 free; enc drain relu(C+bc) for group g+1 on 2/3 of
         groups (the other 1/3 drain on DVE - ACT is the saturated
         engine and DVE has ~300ns slack; ratio-swept optimum).
  DVE  : scalar_tensor_tensor P0=(F0+bf)*E (PSUM drain + bias +
         product in one pass); per-sample tensor_scalar(max,0)+accum
         -> num (4x bf16; relu legal inside the sum since E>0 makes
         relu(F+bf)*E == relu((F+bf)*E)); dens 0-2 via
         tensor_scalar(mult,1)+accum.
  Pool : P1 = FR1 * E pair product (plain tensor_tensor, SBUF bf16 -
         the only useful op it is allowed).
The last group routes pair-1 through a second DVE STT and its dens
through ACT (tail surgery: the serial Pool TT pair and the DVE accum
backlog were on the end-of-kernel critical path). Dens are emitted
before nums on DVE (frees the E tile rotation sooner), and the startup
DMAs go [weights-bf16, nb0, weights-f32] so the first enc chain starts
~0.7us earlier.
Steady state ~2.45us/group; sim 86.1us/core; HW rel err 1.078e-3.
"""

import numpy as np

B, N, D, LAT, EC = 1024, 256, 64, 128, 32
M = 8           # cores
S = B // M      # samples per core (128)
G = S // 4      # main-loop iterations per core (4 samples each)

_CACHE = {}


def _build_bass():
    import concourse.bacc as bacc
    import concourse.tile as tile
    from concourse import mybir

    f32 = mybir.dt.float32
    bf16 = mybir.dt.bfloat16
    AF = mybir.ActivationFunctionType
    ALU = mybir.AluOpType

    nc = bacc.Bacc("TRN2", target_bir_lowering=False)

    nbrT_d = nc.dram_tensor("nbrT", [S * D, N], bf16, kind="ExternalInput")
    # bf16 matmul weights: [0:128) WfH0 | [128:256) WfH1 | [256:320) WcS2
    #   | [320+128j : 448+128j) WaJ (Wa2 in rows 32j), j=0..3
    wtsb_d = nc.dram_tensor("wtsb", [128, 832], bf16, kind="ExternalInput")
    # f32 pieces: [0:128) Wl | 128 bf | 129 bcS2 | [130:258) bl row0
    wtsf_d = nc.dram_tensor("wtsf", [128, 258], f32, kind="ExternalInput")
    out_d = nc.dram_tensor("out", [S, LAT], f32, kind="ExternalOutput")

    with tile.TileContext(nc) as tc:
        from contextlib import ExitStack

        with ExitStack() as ctx:
            singles = ctx.enter_context(tc.tile_pool(name="singles", bufs=1))
            nbr_pool = ctx.enter_context(tc.tile_pool(name="nbr", bufs=4))
            enc_pool = ctx.enter_context(tc.tile_pool(name="enc", bufs=3))
            e_pool = ctx.enter_context(tc.tile_pool(name="e", bufs=4))
            p_pool = ctx.enter_context(tc.tile_pool(name="p", bufs=7))
            # PSUM: A 2x2 banks + F 3x1 + C 1x1 = 8
            ps_att = ctx.enter_context(
                tc.tile_pool(name="ps_att", bufs=2, space="PSUM")
            )
            ps_feat = ctx.enter_context(
                tc.tile_pool(name="ps_feat", bufs=3, space="PSUM")
            )
            ps_comm = ctx.enter_context(
                tc.tile_pool(name="ps_comm", bufs=1, space="PSUM")
            )

            wtsb = singles.tile([128, 832], bf16)
            nc.sync.dma_start(out=wtsb, in_=wtsb_d[:, :])
            nb0_early = nbr_pool.tile([128, 2, N], bf16, tag="nbr")
            nc.sync.dma_start(
                out=nb0_early,
                in_=nbrT_d[0:256, :].rearrange("(t p) n -> p t n", p=128),
            )
            wtsf = singles.tile([128, 258], f32)
            nc.sync.dma_start(out=wtsf, in_=wtsf_d[:, :])
            wfh = [wtsb[:, 0:128], wtsb[:, 128:256]]
            wc_sb = wtsb[:, 256:320]
            waj = [wtsb[:, 320 + 128 * j : 448 + 128 * j] for j in range(4)]
            wl_sb = wtsf[:, 0:128]
            bf_sb = wtsf[:, 128:129]
            bc_sb = wtsf[:, 129:130]
            bl_sb = wtsf[0:1, 130:258]
            ones1 = singles.tile([1, LAT], f32)
            nc.vector.memset(ones1, 1.0)

            den_blk = singles.tile([LAT, S], f32)
            num_blk = singles.tile([LAT, S], f32)
            scr_d = singles.tile([128, N], bf16)   # DVE accum dump
            scr_p = singles.tile([128, N], bf16)   # Pool accum dump

            nbrT_ap = nbrT_d[:, :]
            C2 = ps_comm.tile([128, 2, N], f32, tag="C")

            def load_nb(g):
                r0 = 4 * g * D
                nb = nbr_pool.tile([128, 2, N], bf16, tag="nbr")
                nc.sync.dma_start(
                    out=nb,
                    in_=nbrT_ap[r0 : r0 + 256, :].rearrange(
                        "(t p) n -> p t n", p=128
                    ),
                )
                return nb

            def enc_stage(g, nb):
                C = C2[:, g % 2, :]
                nc.tensor.matmul(
                    out=C[0:64, :], lhsT=wc_sb, rhs=nb[:, 0, :],
                    start=True, stop=True,
                )
                nc.tensor.matmul(
                    out=C[64:128, :], lhsT=wc_sb, rhs=nb[:, 1, :],
                    start=True, stop=True,
                )
                enc = enc_pool.tile([128, N], bf16, tag="enc")
                # alternate the drain engine to balance ACT vs DVE load
                if not (g % 3 == 2):
                    nc.scalar.activation(
                        out=enc, in_=C, func=AF.Relu, bias=bc_sb, scale=1.0
                    )
                else:
                    nc.vector.tensor_scalar(
                        out=enc, in0=C, scalar1=bc_sb, scalar2=0.0,
                        op0=ALU.add, op1=ALU.max,
                    )
                return enc

            nbs = {0: nb0_early, 1: load_nb(1)}
            encs = {0: enc_stage(0, nbs[0])}

            for g in range(G):
                nb = nbs.pop(g)
                if g + 2 < G:
                    nbs[g + 2] = load_nb(g + 2)
                if g + 1 < G:
                    encs[g + 1] = enc_stage(g + 1, nbs[g + 1])
                enc = encs.pop(g)

                # att logits: one [128,4,256] 2-bank PSUM tile
                A = ps_att.tile([128, 4, N], f32, tag="A")
                for j in range(4):
                    nc.tensor.matmul(
                        out=A[:, j, :], lhsT=waj[j], rhs=enc,
                        start=True, stop=True,
                    )

                # feat: per-pair 1-bank PSUM tiles
                F = []
                for t in range(2):
                    Ft = ps_feat.tile([128, 2, N], f32, tag="F")
                    for h in range(2):
                        nc.tensor.matmul(
                            out=Ft[:, h, :], lhsT=wfh[h], rhs=nb[:, t, :],
                            start=True, stop=True,
                        )
                    F.append(Ft)

                last = g == G - 1
                if not last:
                    # pair1 drain on ACT first: it needs only F1 (not E) and
                    # draining it early frees the F buffer for the next
                    # group's feat matmuls (keeps the PE queue moving)
                    FR1 = p_pool.tile([128, 2, N], bf16, tag="p")
                    nc.scalar.activation(
                        out=FR1, in_=F[1], func=AF.Relu, bias=bf_sb, scale=1.0
                    )

                # exp: 3-sample batch + 1-sample with fused den accum
                E = e_pool.tile([128, 4, N], bf16, tag="e")
                nc.scalar.activation(out=E[:, 0:3, :], in_=A[:, 0:3, :], func=AF.Exp)
                nc.scalar.activation(
                    out=E[:, 3, :], in_=A[:, 3, :], func=AF.Exp,
                    accum_out=den_blk[:, 4 * g + 3 : 4 * g + 4],
                )

                # pair0 drain+product on DVE (fused STT)
                P0 = p_pool.tile([128, 2, N], bf16, tag="p")
                nc.vector.scalar_tensor_tensor(
                    out=P0, in0=F[0], scalar=bf_sb,
                    in1=E[:, 0:2, :],
                    op0=ALU.add, op1=ALU.mult,
                )
                P1 = p_pool.tile([128, 2, N], bf16, tag="p")
                if not last:
                    nc.gpsimd.tensor_tensor(
                        out=P1, in0=FR1, in1=E[:, 2:4, :], op=ALU.mult
                    )
                else:
                    # tail surgery: the serial Pool TT would sit on the end
                    # critical path; use a second DVE STT instead and move
                    # this group's dens to the now-idle ACT
                    nc.vector.scalar_tensor_tensor(
                        out=P1, in0=F[1], scalar=bf_sb,
                        in1=E[:, 2:4, :],
                        op0=ALU.add, op1=ALU.mult,
                    )
                P = [P0, P1]

                # per-sample accums: num = sum relu(P) on DVE; dens on DVE
                # except the last group (ACT, overlapping the DVE tail)
                for j in (0, 1, 2):
                    s = 4 * g + j
                    if not last:
                        nc.vector.tensor_scalar(
                            out=scr_d, in0=E[:, j, :],
                            scalar1=1.0, scalar2=None,
                            op0=ALU.mult, op1=ALU.add,
                            accum_out=den_blk[:, s : s + 1],
                        )
                    else:
                        nc.scalar.activation(
                            out=scr_p, in_=E[:, j, :], func=AF.Relu,
                            accum_out=den_blk[:, s : s + 1],
                        )
                for j in (0, 1):
                    s = 4 * g + j
                    nc.vector.tensor_scalar(
                        out=scr_d, in0=P[0][:, j, :],
                        scalar1=0.0, scalar2=None,
                        op0=ALU.max, op1=ALU.add,
                        accum_out=num_blk[:, s : s + 1],
                    )
                for j in (2, 3):
                    s = 4 * g + j
                    nc.vector.tensor_scalar(
                        out=scr_d, in0=P[1][:, j - 2, :],
                        scalar1=0.0, scalar2=None,
                        op0=ALU.max, op1=ALU.add,
                        accum_out=num_blk[:, s : s + 1],
                    )


            # finale: agg = num/den, out = relu(agg @ Wl + bl)
            rden = singles.tile([LAT, S], f32)
            nc.vector.reciprocal(out=rden, in_=den_blk)
            aggT = singles.tile([LAT, S], f32)
            nc.vector.tensor_mul(out=aggT, in0=num_blk, in1=rden)

            out_ps_t = ps_feat.tile([128, 2, N], f32, tag="F")
            out_ps = out_ps_t[:, 0, 0:LAT]
            nc.tensor.matmul(
                out=out_ps, lhsT=aggT, rhs=wl_sb, start=True, stop=False
            )
            nc.tensor.matmul(
                out=out_ps, lhsT=ones1, rhs=bl_sb, start=False, stop=True
            )
            out_sb = singles.tile([S, LAT], f32)
            nc.scalar.activation(out=out_sb, in_=out_ps, func=AF.Relu)
            nc.sync.dma_start(out=out_d[:, :], in_=out_sb)

    nc.finalize()
    return nc


def _host_prep(inputs):
    import ml_dtypes

    bf = np.asarray(inputs["bf"], dtype=np.float32)
    bc = np.asarray(inputs["bc"], dtype=np.float32)
    bl = np.asarray(inputs["bl"], dtype=np.float32)
    Wf = np.asarray(inputs["Wf"], dtype=np.float32)
    Wc = np.asarray(inputs["Wc"], dtype=np.float32)
    Wa = np.asarray(inputs["Wa"], dtype=np.float32)
    Wl = np.asarray(inputs["Wl"], dtype=np.float32)
    Wa2 = Wa[EC : 2 * EC]  # only the enc_comm block survives the softmax shift

    bft = ml_dtypes.bfloat16
    nbr = np.asarray(inputs["neighbor_data"], dtype=np.float32)
    # [M, S, N, D] -> [M, S, D, N] -> [M, S*D, N], cast bf16
    nbrT = (
        np.ascontiguousarray(nbr.reshape(M, S, N, D).transpose(0, 1, 3, 2))
        .reshape(M, S * D, N)
        .astype(bft)
    )

    wtsb = np.zeros((128, 832), dtype=bft)
    wtsb[0:64, 0:128] = Wf.astype(bft)
    wtsb[64:128, 128:256] = Wf.astype(bft)
    wtsb[0:64, 256:288] = Wc.astype(bft)
    wtsb[64:128, 288:320] = Wc.astype(bft)
    for j in range(4):
        wtsb[32 * j : 32 * j + 32, 320 + 128 * j : 448 + 128 * j] = (
            Wa2.astype(bft)
        )

    wtsf = np.zeros((128, 258), dtype=np.float32)
    wtsf[:, 0:128] = Wl
    wtsf[:, 128] = bf
    wtsf[:, 129] = np.tile(bc, 4)
    wtsf[0, 130:258] = bl

    return [{"nbrT": nbrT[c], "wtsb": wtsb, "wtsf": wtsf} for c in range(M)]


def kernel(**inputs) -> np.ndarray:
    from concourse.bass_utils import run_bass_kernel_spmd

    if "nc" not in _CACHE:
        _CACHE["nc"] = _build_bass()
    nc = _CACHE["nc"]

    in_maps = _host_prep(inputs)
    res = run_bass_kernel_spmd(nc, in_maps, list(range(M)))
    out = np.concatenate(
        [np.asarray(res.results[c]["out"]) for c in range(M)], axis=0
    )
    return out.astype(np.float32)


# revision 12
# speedup vs baseline: 1.0326x; 1.0326x over previous
"""Trainium2 Bass kernel for AttentiveRelationalModuleUniformObs, v5.

Math (per sample b over N=256 neighbors, D=64, LAT=128, EC=32):
    feat   = relu(nbr @ Wf + bf)            [N, LAT]
    enc    = relu(nbr @ Wc + bc)            [N, EC]
    att    = softmax_N(enc @ Wa2)           [N, LAT]   (self/mean/ba cancel)
    out[b] = relu((att * feat).sum(N) @ Wl + bl)

Strategy: transposed per-sample layout [LAT on partitions, N on free],
4 samples per group, data-parallel over 8 cores (128 samples/core).
bf16 everywhere except PSUM and accumulators (HW rel err ~1.1e-3).

Engine plan per group (hard ISA limits: GPSIMD cannot touch PSUM and
cannot run accum_out/TensorScalarPtr, so PSUM drains live on ACT/DVE
and all accum reductions on DVE; DVE pays ~95ns pipeline-drain per op,
so op COUNT is what matters there):
  PE   : enc 2x K=128 blockdiag matmuls -> C [128,256] (4 samples' EC
         stacked on partitions, software-pipelined ONE GROUP AHEAD with
         C manually double-buffered inside one PSUM bank), att 4x K=64
         (Wa2 in lhsT rows 32j), feat 4x K=64.
  ACT  : FR1 = relu(F1+bf) FIRST (needs no E; frees the F buffer so
         the next group's feat matmuls aren't blocked); then one
         3-sample batched exp + a 1-sample exp whose accum_out gives
         den3 free; enc drain relu(C+bc) for group g+1 on 2/3 of
         groups (the other 1/3 drain on DVE - ACT is the saturated
         engine and DVE has ~300ns slack; ratio-swept optimum).
  DVE  : scalar_tensor_tensor P0=(F0+bf)*E (PSUM drain + bias +
         product in one pass); per-sample tensor_scalar(max,0)+accum
         -> num (4x bf16; relu legal inside the sum since E>0 makes
         relu(F+bf)*E == relu((F+bf)*E)); dens 0-2 via
         tensor_scalar(mult,1)+accum.
  Pool : P1 = FR1 * E pair product (plain tensor_tensor, SBUF bf16 -
         the only useful op it is allowed).
The last group routes pair-1 through a second DVE STT and its dens
through ACT (tail surgery: the serial Pool TT pair and the DVE accum
backlog were on the end-of-kernel critical path). Dens are emitted
before nums on DVE (frees the E tile rotation sooner), and the startup
DMAs go [weights-bf16, nb0, weights-f32] so the first enc chain starts
~0.7us earlier.
Steady state ~2.45us/group; sim 86.1us/core; HW rel err 1.078e-3.
"""

import numpy as np

B, N, D, LAT, EC = 1024, 256, 64, 128, 32
M = 8           # cores
S = B // M      # samples per core (128)
G = S // 4      # main-loop iterations per core (4 samples each)

_CACHE = {}


def _build_bass():
    import concourse.bacc as bacc
    import concourse.tile as tile
    from concourse import mybir

    f32 = mybir.dt.float32
    bf16 = mybir.dt.bfloat16
    AF = mybir.ActivationFunctionType
    ALU = mybir.AluOpType

    nc = bacc.Bacc("TRN2", target_bir_lowering=False)

    nbrT_d = nc.dram_tensor("nbrT", [S * D, N], bf16, kind="ExternalInput")
    # bf16 matmul weights: [0:128) WfH0 | [128:256) WfH1 | [256:320) WcS2
    #   | [320+128j : 448+128j) WaJ (Wa2 in rows 32j), j=0..3
    wtsb_d = nc.dram_tensor("wtsb", [128, 832], bf16, kind="ExternalInput")
    # f32 pieces: [0:128) Wl | 128 bf | 129 bcS2 | [130:258) bl row0
    wtsf_d = nc.dram_tensor("wtsf", [128, 258], f32, kind="ExternalInput")
    out_d = nc.dram_tensor("out", [S, LAT], f32, kind="ExternalOutput")

    with tile.TileContext(nc) as tc:
        from contextlib import ExitStack

        with ExitStack() as ctx:
            singles = ctx.enter_context(tc.tile_pool(name="singles", bufs=1))
            nbr_pool = ctx.enter_context(tc.tile_pool(name="nbr", bufs=4))
            enc_pool = ctx.enter_context(tc.tile_pool(name="enc", bufs=3))
            e_pool = ctx.enter_context(tc.tile_pool(name="e", bufs=4))
            p_pool = ctx.enter_context(tc.tile_pool(name="p", bufs=7))
            # PSUM: A 2x2 banks + F 3x1 + C 1x1 = 8
            ps_att = ctx.enter_context(
                tc.tile_pool(name="ps_att", bufs=2, space="PSUM")
            )
            ps_feat = ctx.enter_context(
                tc.tile_pool(name="ps_feat", bufs=3, space="PSUM")
            )
            ps_comm = ctx.enter_context(
                tc.tile_pool(name="ps_comm", bufs=1, space="PSUM")
            )

            wtsb = singles.tile([128, 832], bf16)
            nc.sync.dma_start(out=wtsb, in_=wtsb_d[:, :])
            nb0_early = nbr_pool.tile([128, 2, N], bf16, tag="nbr")
            nc.sync.dma_start(
                out=nb0_early,
                in_=nbrT_d[0:256, :].rearrange("(t p) n -> p t n", p=128),
            )
            wtsf = singles.tile([128, 258], f32)
            nc.sync.dma_start(out=wtsf, in_=wtsf_d[:, :])
            wfh = [wtsb[:, 0:128], wtsb[:, 128:256]]
            wc_sb = wtsb[:, 256:320]
            waj = [wtsb[:, 320 + 128 * j : 448 + 128 * j] for j in range(4)]
            wl_sb = wtsf[:, 0:128]
            bf_sb = wtsf[:, 128:129]
            bc_sb = wtsf[:, 129:130]
            bl_sb = wtsf[0:1, 130:258]
            ones1 = singles.tile([1, LAT], f32)
            nc.vector.memset(ones1, 1.0)

            # HAM warmup: the PE is idle during the ~3.5us startup DMA
            # wait, and the clock gate needs ~3.4us of sustained activity
            # to unthrottle 1.2->2.4GHz. Burn the wait on dummy matmuls so
            # the first real groups run at the warm clock.
            warm_src = singles.tile([128, 64], bf16)
            nc.vector.memset(warm_src, 0.0)
            warm_ps = ps_att.tile([128, 4, N], f32, tag="A")
            for _ in range(30):
                nc.tensor.matmul(
                    out=warm_ps[0:64, 0, 0:64], lhsT=warm_src, rhs=warm_src,
                    start=True, stop=True,
                )

            den_blk = singles.tile([LAT, S], f32)
            num_blk = singles.tile([LAT, S], f32)
            scr_d = singles.tile([128, N], bf16)   # DVE accum dump
            scr_p = singles.tile([128, N], bf16)   # Pool accum dump

            nbrT_ap = nbrT_d[:, :]
            C2 = ps_comm.tile([128, 2, N], f32, tag="C")

            def load_nb(g):
                r0 = 4 * g * D
                nb = nbr_pool.tile([128, 2, N], bf16, tag="nbr")
                nc.sync.dma_start(
                    out=nb,
                    in_=nbrT_ap[r0 : r0 + 256, :].rearrange(
                        "(t p) n -> p t n", p=128
                    ),
                )
                return nb

            def enc_stage(g, nb):
                C = C2[:, g % 2, :]
                nc.tensor.matmul(
                    out=C[0:64, :], lhsT=wc_sb, rhs=nb[:, 0, :],
                    start=True, stop=True,
                )
                nc.tensor.matmul(
                    out=C[64:128, :], lhsT=wc_sb, rhs=nb[:, 1, :],
                    start=True, stop=True,
                )
                enc = enc_pool.tile([128, N], bf16, tag="enc")
                # alternate the drain engine to balance ACT vs DVE load
                if not (g % 3 == 2):
                    nc.scalar.activation(
                        out=enc, in_=C, func=AF.Relu, bias=bc_sb, scale=1.0
                    )
                else:
                    nc.vector.tensor_scalar(
                        out=enc, in0=C, scalar1=bc_sb, scalar2=0.0,
                        op0=ALU.add, op1=ALU.max,
                    )
                return enc

            nbs = {0: nb0_early, 1: load_nb(1)}
            encs = {0: enc_stage(0, nbs[0])}

            for g in range(G):
                nb = nbs.pop(g)
                if g + 2 < G:
                    nbs[g + 2] = load_nb(g + 2)
                if g + 1 < G:
                    encs[g + 1] = enc_stage(g + 1, nbs[g + 1])
                enc = encs.pop(g)

                # att logits: one [128,4,256] 2-bank PSUM tile
                A = ps_att.tile([128, 4, N], f32, tag="A")
                for j in range(4):
                    nc.tensor.matmul(
                        out=A[:, j, :], lhsT=waj[j], rhs=enc,
                        start=True, stop=True,
                    )

                # feat: per-pair 1-bank PSUM tiles
                F = []
                for t in range(2):
                    Ft = ps_feat.tile([128, 2, N], f32, tag="F")
                    for h in range(2):
                        nc.tensor.matmul(
                            out=Ft[:, h, :], lhsT=wfh[h], rhs=nb[:, t, :],
                            start=True, stop=True,
                        )
                    F.append(Ft)

                last = g == G - 1
                if not last:
                    # pair1 drain on ACT first: it needs only F1 (not E) and
                    # draining it early frees the F buffer for the next
                    # group's feat matmuls (keeps the PE queue moving)
                    FR1 = p_pool.tile([128, 2, N], bf16, tag="p")
                    nc.scalar.activation(
                        out=FR1, in_=F[1], func=AF.Relu, bias=bf_sb, scale=1.0
                    )

                # exp: 3-sample batch + 1-sample with fused den accum
                E = e_pool.tile([128, 4, N], bf16, tag="e")
                nc.scalar.activation(out=E[:, 0:3, :], in_=A[:, 0:3, :], func=AF.Exp)
                nc.scalar.activation(
                    out=E[:, 3, :], in_=A[:, 3, :], func=AF.Exp,
                    accum_out=den_blk[:, 4 * g + 3 : 4 * g + 4],
                )

                # pair0 drain+product on DVE (fused STT)
                P0 = p_pool.tile([128, 2, N], bf16, tag="p")
                nc.vector.scalar_tensor_tensor(
                    out=P0, in0=F[0], scalar=bf_sb,
                    in1=E[:, 0:2, :],
                    op0=ALU.add, op1=ALU.mult,
                )
                P1 = p_pool.tile([128, 2, N], bf16, tag="p")
                if not last:
                    nc.gpsimd.tensor_tensor(
                        out=P1, in0=FR1, in1=E[:, 2:4, :], op=ALU.mult
                    )
                else:
                    # tail surgery: the serial Pool TT would sit on the end
                    # critical path; use a second DVE STT instead and move
                    # this group's dens to the now-idle ACT
                    nc.vector.scalar_tensor_tensor(
                        out=P1, in0=F[1], scalar=bf_sb,
                        in1=E[:, 2:4, :],
                        op0=ALU.add, op1=ALU.mult,
                    )
                P = [P0, P1]

                # per-sample accums: num = sum relu(P) on DVE; dens on DVE
                # except the last group (ACT, overlapping the DVE tail)
                for j in (0, 1, 2):
                    s = 4 * g + j
                    if not last:
                        nc.vector.tensor_scalar(
                            out=scr_d, in0=E[:, j, :],
                            scalar1=1.0, scalar2=None,
                            op0=ALU.mult, op1=ALU.add,
                            accum_out=den_blk[:, s : s + 1],
                        )
                    else:
                        nc.scalar.activation(
                            out=scr_p, in_=E[:, j, :], func=AF.Relu,
                            accum_out=den_blk[:, s : s + 1],
                        )
                for j in (0, 1):
                    s = 4 * g + j
                    nc.vector.tensor_scalar(
                        out=scr_d, in0=P[0][:, j, :],
                        scalar1=0.0, scalar2=None,
                        op0=ALU.max, op1=ALU.add,
                        accum_out=num_blk[:, s : s + 1],
                    )
                for j in (2, 3):
                    s = 4 * g + j
                    nc.vector.tensor_scalar(
                        out=scr_d, in0=P[1][:, j - 2, :],
                        scalar1=0.0, scalar2=None,
                        op0=ALU.max, op1=ALU.add,
                        accum_out=num_blk[:, s : s + 1],
                    )


            # finale: agg = num/den, out = relu(agg @ Wl + bl)
            rden = singles.tile([LAT, S], f32)
            nc.vector.reciprocal(out=rden, in_=den_blk)
            aggT = singles.tile([LAT, S], f32)
            nc.vector.tensor_mul(out=aggT, in0=num_blk, in1=rden)

            out_ps_t = ps_feat.tile([128, 2, N], f32, tag="F")
            out_ps = out_ps_t[:, 0, 0:LAT]
            nc.tensor.matmul(
                out=out_ps, lhsT=aggT, rhs=wl_sb, start=True, stop=False
            )
            nc.tensor.matmul(
                out=out_ps, lhsT=ones1, rhs=bl_sb, start=False, stop=True
            )
            out_sb = singles.tile([S, LAT], f32)
            nc.scalar.activation(out=out_sb, in_=out_ps, func=AF.Relu)
            nc.sync.dma_start(out=out_d[:, :], in_=out_sb)

    nc.finalize()
    return nc


def _host_prep(inputs):
    import ml_dtypes

    bf = np.asarray(inputs["bf"], dtype=np.float32)
    bc = np.asarray(inputs["bc"], dtype=np.float32)
    bl = np.asarray(inputs["bl"], dtype=np.float32)
    Wf = np.asarray(inputs["Wf"], dtype=np.float32)
    Wc = np.asarray(inputs["Wc"], dtype=np.float32)
    Wa = np.asarray(inputs["Wa"], dtype=np.float32)
    Wl = np.asarray(inputs["Wl"], dtype=np.float32)
    Wa2 = Wa[EC : 2 * EC]  # only the enc_comm block survives the softmax shift

    bft = ml_dtypes.bfloat16
    nbr = np.asarray(inputs["neighbor_data"], dtype=np.float32)
    # [M, S, N, D] -> [M, S, D, N] -> [M, S*D, N], cast bf16
    nbrT = (
        np.ascontiguousarray(nbr.reshape(M, S, N, D).transpose(0, 1, 3, 2))
        .reshape(M, S * D, N)
        .astype(bft)
    )

    wtsb = np.zeros((128, 832), dtype=bft)
    wtsb[0:64, 0:128] = Wf.astype(bft)
    wtsb[64:128, 128:256] = Wf.astype(bft)
    wtsb[0:64, 256:288] = Wc.astype(bft)
    wtsb[64:128, 288:320] = Wc.astype(bft)
    for j in range(4):
        wtsb[32 * j : 32 * j + 32, 320 + 128 * j : 448 + 128 * j] = (
            Wa2.astype(bft)
        )

    wtsf = np.zeros((128, 258), dtype=np.float32)
    wtsf[:, 0:128] = Wl
    wtsf[:, 128] = bf
    wtsf[:, 129] = np.tile(bc, 4)
    wtsf[0, 130:258] = bl

    return [{"nbrT": nbrT[c], "wtsb": wtsb, "wtsf": wtsf} for c in range(M)]


def kernel(**inputs) -> np.ndarray:
    from concourse.bass_utils import run_bass_kernel_spmd

    if "nc" not in _CACHE:
        _CACHE["nc"] = _build_bass()
    nc = _CACHE["nc"]

    in_maps = _host_prep(inputs)
    res = run_bass_kernel_spmd(nc, in_maps, list(range(M)))
    out = np.concatenate(
        [np.asarray(res.results[c]["out"]) for c in range(M)], axis=0
    )
    return out.astype(np.float32)
